# revision 42
# baseline (speedup 1.0000x reference)
"""Trainium2 Bass kernel for BertSelfAttention-style block (post-norm, shared-V).

Reference computation (per batch b, N=1024 tokens, d=256, H=8 heads):
  Q = x @ Wq.T   (N, H*d) ; K = x @ Wk.T
  scores = Q_h @ K_h.T / sqrt(d)        per head
  attn = softmax(scores, axis=-1)       -> output #2 (b, 32, 32, H, 32, 32)
  ctx_h = attn_h @ x                    (shared V = x)
  logits = concat_h(ctx_h) @ Wv.T       (N, d)
  out = LayerNorm(logits + x) * gamma + beta -> output #1 (b, 32, 32, d)

Sharding: 8 cores = 4 batches x 2 query-slices of 512 tokens. Each core
computes its 512 queries fully (K/V over all 1024 tokens of its batch),
including softmax, context, Wv projection and layernorm. No collectives.

Matmuls run in float32r (full-rate PE fp32 mode, fp32 PSUM accumulation);
walrus requires every matmul operand's producer to emit fp32r-rounded
values, so all matmul-operand tiles are allocated as float32r.
"""

import sys

if "/opt/trn_rl_repo" not in sys.path:
    sys.path.insert(0, "/opt/trn_rl_repo")

import numpy as np

import concourse.bass as bass
import concourse.tile as tile
from concourse import mybir
from concourse.masks import make_identity
from concourse.vector_clock import ScopedClock

F32 = mybir.dt.float32
MM_DT = mybir.dt.float32r
BF16 = mybir.dt.bfloat16

HEADS = 8
D = 256
N = 1024  # keys per batch
NQ = 512  # queries per core
LN_EPS = 1e-5
P = 128

_DRAIN_MAXW = 1


def _patched_drain_and_barrier(self, tick_clock, wait_clock):
    """TileContext exit drain, with sem waits chunked onto nofuse nops.

    The walrus build in this container rejects instructions carrying more
    than one sync wait, and the stock exit drain waits on every live
    processor at once.
    """
    nc = self.nc
    nop = nc.sync.nop(nofuse=True)
    wait_clock.add_sem_waits(nop.ins, ScopedClock({None: tick_clock.global_clock}))
    si = nop.ins.sync_info
    waits = list(si.on_wait) if si is not None else []
    if len(waits) > _DRAIN_MAXW:
        nop.ins.sync_info = mybir.SyncInfo(on_wait=waits[:_DRAIN_MAXW], on_update=[])
        for i in range(_DRAIN_MAXW, len(waits), _DRAIN_MAXW):
            n2 = nc.sync.nop(nofuse=True)
            n2.ins.sync_info = mybir.SyncInfo(
                on_wait=waits[i : i + _DRAIN_MAXW], on_update=[]
            )
    nc.sync.drain()
    nc.all_engine_barrier()
    popped = nc._tile_sem_poison_stack.pop()
    assert popped is self._sem_poison
    nc.clear_and_free_semaphores(list(self.sems.allocated().values()))
    nc.all_engine_barrier()


tile.TileContext._drain_and_barrier = _patched_drain_and_barrier


def legalize_sync_waits(nc, limit=1):
    """Hoist excess per-instruction sync waits onto nofuse nops placed just
    before, on the same engine.  Waiting earlier on the same engine is always
    safe (the producers of those sems are never gated on this engine's
    progress past this point)."""
    n_split = 0
    for fn in nc.m.functions:
        for blk in fn.blocks:
            new_list = []
            for inst in blk.instructions:
                si = inst.sync_info
                waits = list(si.on_wait) if si is not None else []
                if len(waits) > limit:
                    excess, keep = waits[:-limit], waits[-limit:]
                    for w in excess:
                        nop = mybir.InstNoOp(
                            name=nc.get_next_instruction_name(), ins=[], outs=[]
                        )
                        nop.engine = inst.engine
                        nop.bass_nofuse = True
                        nop.sync_info = mybir.SyncInfo(on_wait=[w], on_update=[])
                        nc.register_instruction(nop)
                        new_list.append(nop)
                        n_split += 1
                    inst.sync_info = mybir.SyncInfo(
                        on_wait=keep, on_update=list(si.on_update)
                    )
                new_list.append(inst)
            blk.instructions[:] = new_list
    return n_split


def build_program():
    nc = bass.Bass()
    xq = nc.dram_tensor("xq", [NQ, D], F32, kind="ExternalInput")
    xk = nc.dram_tensor("xk", [N, D], F32, kind="ExternalInput")
    Wq = nc.dram_tensor("Wq", [HEADS * D, D], F32, kind="ExternalInput")
    Wk = nc.dram_tensor("Wk", [HEADS * D, D], F32, kind="ExternalInput")
    Wv = nc.dram_tensor("Wv", [D, HEADS * D], F32, kind="ExternalInput")
    gamma = nc.dram_tensor("gamma", [D], F32, kind="ExternalInput")
    beta = nc.dram_tensor("beta", [D], F32, kind="ExternalInput")
    out = nc.dram_tensor("out", [NQ, D], F32, kind="ExternalOutput")
    probs = nc.dram_tensor("probs", [NQ, HEADS * N], F32, kind="ExternalOutput")

    xq_r = xq.rearrange("(t p) d -> p t d", p=P)  # [128, 4, 256]
    xk_r = xk.rearrange("(t p) d -> p t d", p=P)  # [128, 8, 256]
    Wq_r = Wq.rearrange("(t p) d -> p t d", p=P)  # [128, 16, 256]
    Wk_r = Wk.rearrange("(t p) d -> p t d", p=P)
    Wv_r = Wv.rearrange("(t p) e -> p t e", p=P)  # [128, 2, 2048]
    probs_r = probs.rearrange("(t p) m -> p t m", p=P)  # [128, 4, 8192]
    out_r = out.rearrange("(t p) d -> p t d", p=P)  # [128, 4, 256]

    NT = NQ // P  # 4 query row-tiles
    MT = N // P  # 8 key row-tiles
    Exp = mybir.ActivationFunctionType.Exp
    Sqrt = mybir.ActivationFunctionType.Sqrt

    with tile.TileContext(nc) as tc:
        with (
            tc.tile_pool(name="persist", bufs=1) as pp,
            tc.tile_pool(name="wtmp", bufs=4) as wtmp,
            tc.tile_pool(name="wT", bufs=2) as wTp,
            tc.tile_pool(name="qk", bufs=2) as qkp,
            tc.tile_pool(name="escore", bufs=4) as ep,
            tc.tile_pool(name="aout", bufs=2) as ap_,
            tc.tile_pool(name="small", bufs=2) as sp,
            tc.tile_pool(name="ln", bufs=2) as lp,
            tc.tile_pool(name="ps1", bufs=4, space="PSUM") as ps1,
            tc.tile_pool(name="ps2", bufs=2, space="PSUM") as ps2,
            tc.tile_pool(name="dscratch", bufs=2, space="DRAM") as dp,
        ):
            # ---- persistent tiles ----
            xq_sb = pp.tile([P, NT, D], F32)  # queries, natural [n, d]
            x_m = pp.tile([P, MT, D], BF16)  # bf16 keys for ctx matmul
            xqT = pp.tile([P, 2, NQ], MM_DT)  # [d, n]
            xkT = pp.tile([P, 2, N], MM_DT)  # [d, m]
            WvT = pp.tile([P, 16, D], BF16)  # [he, dout]
            ctxT = pp.tile([P, 16, NQ], BF16)  # [he, n]
            ident = pp.tile([P, P], F32)
            gammaB = pp.tile([P, D], F32)
            betaB = pp.tile([P, D], F32)
            epsT = pp.tile([P, 1], F32)


            make_identity(nc, ident)
            nc.vector.memset(epsT, LN_EPS)
            nc.sync.dma_start(out=xq_sb, in_=xq_r)
            nc.sync.dma_start(out=gammaB, in_=gamma[None, :].to_broadcast((P, D)))
            nc.sync.dma_start(out=betaB, in_=beta[None, :].to_broadcast((P, D)))

            def pe_T(dst, src):
                """dst[:128, :128] = src.T via PE, evict DVE."""
                ps = ps1.tile([P, 512], F32, tag="ps1")
                nc.tensor.transpose(ps[:, :P], src, ident)
                nc.vector.tensor_copy(dst, ps[:, :P])

            # ---- input transposes (xk streamed per row-tile) ----
            for t in range(NT):
                for db in range(2):
                    pe_T(
                        xqT[:, db, t * P : (t + 1) * P],
                        xq_sb[:, t, db * P : (db + 1) * P],
                    )
            for t in range(MT):
                w = wtmp.tile([P, D], F32, tag="w")
                nc.sync.dma_start(out=w, in_=xk_r[:, t, :])
                nc.vector.tensor_copy(x_m[:, t, :], w)
                for db in range(2):
                    pe_T(
                        xkT[:, db, t * P : (t + 1) * P],
                        w[:, db * P : (db + 1) * P],
                    )
            for t in range(2):
                wv = wtmp.tile([P, HEADS * D // 2], F32, tag="wv", bufs=2)
                nc.sync.dma_start(out=wv, in_=Wv_r[:, t, : HEADS * D // 2])
                for hb in range(8):
                    pe_T(
                        WvT[:, hb, t * P : (t + 1) * P],
                        wv[:, hb * P : (hb + 1) * P],
                    )
                wv2 = wtmp.tile([P, HEADS * D // 2], F32, tag="wv", bufs=2)
                nc.sync.dma_start(out=wv2, in_=Wv_r[:, t, HEADS * D // 2 :])
                for hb in range(8):
                    pe_T(
                        WvT[:, 8 + hb, t * P : (t + 1) * P],
                        wv2[:, hb * P : (hb + 1) * P],
                    )

            # ---- per head, software-pipelined ----
            # Stage A (head h): weight transposes, Q/K projections, scores^T
            # [m, n] + exp -> E^T (bf16), scores [n, m] + exp -> E (f32) +
            # rowsum, probs out.
            # Stage B (head h-1): ctx matmuls from E^T, normalize into ctxT.
            # Stage B's PE work depends only on data finished well before, so
            # the PE stream never stalls waiting on ACT exps.
            prev = None

            def stage_b(pv):
                ph, p_etT, p_recip = pv
                cps = []
                for _c in range(2):
                    cacc = ps1.tile([P, 512], F32, tag="ps1")
                    cps.append(cacc)
                for mt in range(MT):
                    for db2 in range(2):
                        nc.tensor.matmul(
                            cps[db2],
                            x_m[:, mt, db2 * P : (db2 + 1) * P],
                            p_etT[:, mt, :],
                            start=(mt == 0),
                            stop=(mt == MT - 1),
                        )
                ctxU = sp.tile([P, 2, NQ], F32, tag="ctxU")
                for db2 in range(2):
                    nc.vector.tensor_copy(ctxU[:, db2, :], cps[db2])
                # 1/rowsum as free-axis row vector, broadcast over partitions
                rF = sp.tile([1, NQ], F32, tag="rF")
                for nt in range(NT):
                    ps = ps1.tile([P, 512], F32, tag="ps1")
                    nc.tensor.transpose(ps[:1, :P], p_recip[:, nt : nt + 1], ident)
                    nc.vector.tensor_copy(rF[:1, nt * P : (nt + 1) * P], ps[:1, :P])
                rFd = dp.tile([1, NQ], F32, tag="rFd")
                nc.sync.dma_start(out=rFd, in_=rF[:1, :])
                rB = sp.tile([P, NQ], F32, tag="rB")
                nc.sync.dma_start(out=rB, in_=rFd.to_broadcast((P, NQ)))
                for db2 in range(2):
                    nc.vector.tensor_mul(
                        ctxT[:, ph * 2 + db2, :], ctxU[:, db2, :], rB
                    )

            for h in range(HEADS):
                # stream this head's weight rows and transpose to [d, dd]
                wq_h = wtmp.tile([P, 2, D], F32, tag="wq", bufs=2)
                nc.sync.dma_start(out=wq_h, in_=Wq_r[:, 2 * h : 2 * h + 2, :])
                wk_h = wtmp.tile([P, 2, D], F32, tag="wk", bufs=2)
                nc.sync.dma_start(out=wk_h, in_=Wk_r[:, 2 * h : 2 * h + 2, :])
                wqT = wTp.tile([P, 2, D], MM_DT, tag="wqT")  # [d, dd]
                wkT = wTp.tile([P, 2, D], MM_DT, tag="wkT")
                for t2 in range(2):
                    for db in range(2):
                        pe_T(
                            wqT[:, db, t2 * P : (t2 + 1) * P],
                            wq_h[:, t2, db * P : (db + 1) * P],
                        )
                        pe_T(
                            wkT[:, db, t2 * P : (t2 + 1) * P],
                            wk_h[:, t2, db * P : (db + 1) * P],
                        )

                # Q^T_h [dd, n], K^T_h [dd, m]
                qt = qkp.tile([P, 2, NQ], MM_DT, tag="qt")
                kt = qkp.tile([P, 2, N], MM_DT, tag="kt")
                for db2 in range(2):
                    ps = ps1.tile([P, 512], F32, tag="ps1")
                    for db in range(2):
                        nc.tensor.matmul(
                            ps,
                            wqT[:, db, db2 * P : (db2 + 1) * P],
                            xqT[:, db, :],
                            start=(db == 0),
                            stop=(db == 1),
                        )
                    nc.vector.tensor_copy(qt[:, db2, :], ps)
                for db2 in range(2):
                    for mb in range(2):
                        ps = ps1.tile([P, 512], F32, tag="ps1")
                        for db in range(2):
                            nc.tensor.matmul(
                                ps,
                                wkT[:, db, db2 * P : (db2 + 1) * P],
                                xkT[:, db, mb * 512 : (mb + 1) * 512],
                                start=(db == 0),
                                stop=(db == 1),
                            )
                        nc.vector.tensor_copy(kt[:, db2, mb * 512 : (mb + 1) * 512], ps)

                # scores^T [m, n] -> E^T (bf16) for the deferred ctx matmuls
                etT = ep.tile([P, MT, NQ], BF16, tag="etT", bufs=2)
                for mt in range(MT):
                    ps = ps1.tile([P, 512], F32, tag="ps1")
                    for db2 in range(2):
                        nc.tensor.matmul(
                            ps,
                            kt[:, db2, mt * P : (mt + 1) * P],
                            qt[:, db2, :],
                            start=(db2 == 0),
                            stop=(db2 == 1),
                        )
                    nc.scalar.activation(etT[:, mt, :], ps, Exp, scale=0.0625)

                # scores [n, m] -> E (f32) + rowsum -> A -> probs out
                rs = sp.tile([P, NT], F32, tag="rs")
                e_tiles = []
                for nt in range(NT):
                    psc = ps2.tile([P, N], F32, tag="ps2")
                    for mb in range(2):
                        for db2 in range(2):
                            nc.tensor.matmul(
                                psc[:, mb * 512 : (mb + 1) * 512],
                                qt[:, db2, nt * P : (nt + 1) * P],
                                kt[:, db2, mb * 512 : (mb + 1) * 512],
                                start=(db2 == 0),
                                stop=(db2 == 1),
                            )
                    e = ep.tile([P, N], F32, tag="e")
                    nc.scalar.activation(
                        e, psc, Exp, scale=0.0625, accum_out=rs[:, nt : nt + 1]
                    )
                    e_tiles.append(e)
                recip = sp.tile([P, NT], F32, tag="recip")
                nc.vector.reciprocal(recip, rs)
                for nt in range(NT):
                    a = ap_.tile([P, N], F32, tag="a")
                    nc.vector.tensor_scalar_mul(a, e_tiles[nt], recip[:, nt : nt + 1])
                    nc.sync.dma_start(
                        out=probs_r[:, nt, h * N : (h + 1) * N], in_=a
                    )

                if prev is not None:
                    stage_b(prev)
                prev = (h, etT, recip)

            stage_b(prev)

            # ---- Wv projection + residual + layernorm ----
            for nt in range(NT):
                ps = ps1.tile([P, 512], F32, tag="ps1")
                for ht in range(16):
                    nc.tensor.matmul(
                        ps[:, :D],
                        ctxT[:, ht, nt * P : (nt + 1) * P],
                        WvT[:, ht, :],
                        start=(ht == 0),
                        stop=(ht == 15),
                    )
                t0 = lp.tile([P, D], F32, tag="t0")
                nc.vector.tensor_add(t0, ps[:, :D], xq_sb[:, nt, :])
                st = lp.tile([P, 6], F32, tag="st")
                nc.vector.bn_stats(st, t0)
                mv = lp.tile([P, 2], F32, tag="mv")
                nc.vector.bn_aggr(mv, st)
                std = lp.tile([P, 1], F32, tag="std")
                nc.scalar.activation(std, mv[:, 1:2], Sqrt, bias=epsT)
                rstd = lp.tile([P, 1], F32, tag="rstd")
                nc.vector.reciprocal(rstd, std)
                nmb = lp.tile([P, 1], F32, tag="nmb")
                nc.vector.tensor_mul(nmb, mv[:, 0:1], rstd)
                nc.vector.tensor_scalar_mul(nmb, nmb, -1.0)
                xn = lp.tile([P, D], F32, tag="xn")
                nc.vector.tensor_scalar(
                    xn,
                    t0,
                    rstd,
                    nmb,
                    op0=mybir.AluOpType.mult,
                    op1=mybir.AluOpType.add,
                )
                o1 = lp.tile([P, D], F32, tag="o1")
                nc.vector.tensor_mul(o1, xn, gammaB)
                o2 = lp.tile([P, D], F32, tag="o2")
                nc.vector.tensor_add(o2, o1, betaB)
                nc.sync.dma_start(out=out_r[:, nt, :], in_=o2)

    legalize_sync_waits(nc)
    return nc


_NC = None


def _get_nc():
    global _NC
    if _NC is None:
        _NC = build_program()
    return _NC


def _make_in_maps(x, Wq, Wk, Wv, gamma, beta):
    x = np.ascontiguousarray(np.asarray(x, dtype=np.float32))
    Wq = np.ascontiguousarray(np.asarray(Wq, dtype=np.float32))
    Wk = np.ascontiguousarray(np.asarray(Wk, dtype=np.float32))
    Wv = np.ascontiguousarray(np.asarray(Wv, dtype=np.float32))
    gamma = np.ascontiguousarray(np.asarray(gamma, dtype=np.float32))
    beta = np.ascontiguousarray(np.asarray(beta, dtype=np.float32))
    bs, hh, ww, d = x.shape
    xf = x.reshape(bs, hh * ww, d)
    in_maps, shards = [], []
    for b in range(bs):
        for s in range(2):
            shards.append((b, s))
            in_maps.append(
                {
                    "xq": np.ascontiguousarray(xf[b, s * NQ : (s + 1) * NQ]),
                    "xk": np.ascontiguousarray(xf[b]),
                    "Wq": Wq,
                    "Wk": Wk,
                    "Wv": Wv,
                    "gamma": gamma,
                    "beta": beta,
                }
            )
    return in_maps, shards, (bs, hh, ww, d)


def kernel(x, Wq, Wk, Wv, gamma, beta):
    from concourse.bass_utils import run_bass_kernel_spmd

    nc = _get_nc()
    in_maps, shards, (bs, hh, ww, d) = _make_in_maps(x, Wq, Wk, Wv, gamma, beta)
    res = run_bass_kernel_spmd(nc, in_maps, core_ids=list(range(8)))

    out = np.empty((bs, hh * ww, d), dtype=np.float32)
    probs = np.empty((bs, hh * ww, HEADS, N), dtype=np.float32)
    for i, (b, s) in enumerate(shards):
        out[b, s * NQ : (s + 1) * NQ] = res.results[i]["out"]
        probs[b, s * NQ : (s + 1) * NQ] = res.results[i]["probs"].reshape(
            NQ, HEADS, N
        )
    return (
        out.reshape(bs, hh, ww, d),
        probs.reshape(bs, hh, ww, HEADS, hh, ww),
    )


def _install_ntff_hook_shim():
    """The agent image's antenv lacks axon_hooks; register an equivalent
    backed by ctypes calls into libaxon_pjrt.so so trace=True works."""
    import types

    if "antenv.axon_hooks" in sys.modules:
        return
    if "/root/.axon_site" not in sys.path:
        sys.path.insert(0, "/root/.axon_site")
    from trn_agent_boot.trn_boot import _ntff_profile_via_ctypes

    hook = _ntff_profile_via_ctypes("/opt/axon/libaxon_pjrt.so")
    mod = types.ModuleType("antenv.axon_hooks")
    mod.get_axon_ntff_profile_hook = lambda: hook
    mod.set_axon_ntff_profile_hook = lambda h: None
    sys.modules["antenv.axon_hooks"] = mod

    import concourse.bass_utils as bu

    bu.upload_artifacts = lambda tmpdir: "local://" + tmpdir


def profile_exec_ns(inputs, trace_all_cores=False, tmpdir=None):
    """Run once with NTFF tracing; returns (exec_time_ns, BassKernelResults)."""
    _install_ntff_hook_shim()
    from concourse.bass_utils import run_bass_kernel_spmd

    nc = _get_nc()
    in_maps, _, _ = _make_in_maps(**inputs)
    kw = {}
    if trace_all_cores:
        kw["trace_cores"] = list(range(8))
    if tmpdir is not None:
        kw["tmpdir"] = tmpdir
    res = run_bass_kernel_spmd(
        nc, in_maps, core_ids=list(range(8)), trace=True, **kw
    )
    return res.exec_time_ns, res
